# revision 12
# baseline (speedup 1.0000x reference)
"""CrossPixContrastive loss on 8 trn2 NeuronCores.

Math (per batch n, HW=4096, C=256):
  rgb_n = l2norm_C(rgb); ir_n = l2norm_C(ir)
  e[p,q] = exp(20 * clip(<rgb_n[:,p], ir_n[:,q]>, -1, 1))
  S[p] = sum_q e ; M[p] = sum_q e * (rm_p == im_q)
  C[q] = sum_p e ; Mc[q] = sum_p e * (rm_p == im_q)
  r_rgb = M/(S+1e-6) ; r_ir = Mc/(C+1e-6)
  loss = mean(-log over nonzero of concat(r_rgb, r_ir) * fg)

Sharding: 8 cores = 4 batches x 2 halves of the rgb-pixel axis p.
Per-core tiling: [128p x 1024q] tiles of e.
  PE  : fp8(e4m3) DoubleRow matmuls (both 128-c chunks in one instr,
        0.5 cyc/row) for the logits; bf16 one-hot column-sum matmuls;
        window matmuls + column transposes for the norms
  ACT : e = Exp(scale_p * logit) -> bf16, scale_p = (20/16)/||rgb_p||
  DVE : masked accum (im==rm)*e -> M and plain accum e -> S, both at
        the 4x bf16 perf mode; psum drains
  GPS : ir-seg squares/broadcast/normalize-to-fp8 for streamed segs
ir is pre-scaled by 16/||ir_q|| into fp8; rgb is cast raw to fp8 and
its norm is folded into the per-partition ACT scale.
Host combines the tiny per-core partials into the scalar loss.
"""
import numpy as np
import ml_dtypes

import concourse.bacc as bacc
import concourse.tile as tile
from concourse import mybir
from concourse.bass_utils import run_bass_kernel_spmd

dt = mybir.dt
AF = mybir.ActivationFunctionType
ALU = mybir.AluOpType
DR = mybir.MatmulPerfMode.DoubleRow

N, C, H, W = 4, 256, 64, 64
HW = H * W                      # 4096
PH = HW // 2                    # 2048  p-half per core
NPT = PH // 128                 # 16    p-tiles
QB = 1024                       # q big-chunk
NQB = HW // QB                  # 4
NSEG = HW // 512                # 8 ir segs
NCLS = 5
IRS = 16.0                      # fp8 pre-scale on normalized ir
LN_IRS = float(np.log(IRS))
LN_RS = float(np.log(20.0 / IRS))
EPS_DEN = 1e-6
EPS_NSQ = 1e-24                 # clamp on ||x||^2  (=(1e-12)^2)
DEFER = 3                       # col-matmul deferral (tiles)

_CACHED_NC = None

_TABLES_PATCHED = False


def _patch_activation_tables():
    """Keep Exp/Ln only in natural_log_exp_and_others so the compiler
    loads one ACT table set instead of thrashing between exp/ln sets."""
    global _TABLES_PATCHED
    if _TABLES_PATCHED:
        return
    _TABLES_PATCHED = True
    import concourse.hw_specs as hw_specs
    import concourse.bacc as _bacc
    orig = hw_specs.get_activation_tables

    def patched(arch):
        tabs = {k: set(v) for k, v in orig(arch).items()}
        exp, ln = AF.Exp, AF.Ln
        for name, fns in tabs.items():
            if name != "natural_log_exp_and_others":
                fns.discard(exp)
                fns.discard(ln)
        return tabs

    hw_specs.get_activation_tables = patched
    if getattr(_bacc, "get_activation_tables", None) is orig:
        _bacc.get_activation_tables = patched


def build_nc():
    _patch_activation_tables()
    nc = bacc.Bacc("TRN2", target_bir_lowering=False, debug=False, num_devices=8)

    rgb_in = nc.dram_tensor("rgb_half", [2, 128, PH], dt.float32, kind="ExternalInput").ap()
    ir_in = nc.dram_tensor("ir_full", [2, 128, HW], dt.float32, kind="ExternalInput").ap()
    im_in = nc.dram_tensor("im_bcast", [128, HW], dt.bfloat16, kind="ExternalInput").ap()
    rm_in = nc.dram_tensor("rm_cols", [128, NPT], dt.float32, kind="ExternalInput").ap()
    oh_in = nc.dram_tensor("oh_lhsT", [128, NPT * 6], dt.bfloat16, kind="ExternalInput").ap()
    id_in = nc.dram_tensor("ident", [128, 128], dt.float32, kind="ExternalInput").ap()

    S_out = nc.dram_tensor("S_out", [NPT, 128], dt.float32, kind="ExternalOutput").ap()
    M_out = nc.dram_tensor("M_out", [NPT, 128], dt.float32, kind="ExternalOutput").ap()
    C_out = nc.dram_tensor("C_out", [6, HW], dt.float32, kind="ExternalOutput").ap()

    with tile.TileContext(nc) as tc:
        with tc.tile_pool(name="big", bufs=1) as big, \
             tc.tile_pool(name="ldr", bufs=2) as ldr, \
             tc.tile_pool(name="ldi", bufs=4) as ldi, \
             tc.tile_pool(name="scr", bufs=2) as scr, \
             tc.tile_pool(name="rows", bufs=2) as rows, \
             tc.tile_pool(name="epool", bufs=6) as epool, \
             tc.tile_pool(name="psL", bufs=2, space="PSUM") as psL, \
             tc.tile_pool(name="psC", bufs=1, space="PSUM") as psCp, \
             tc.tile_pool(name="psN", bufs=1, space="PSUM") as psN:

            # ---------------- persistent tiles ----------------
            im_b = big.tile([128, HW], dt.bfloat16)
            rm_c = big.tile([128, NPT], dt.float32)
            oh_b = big.tile([128, NPT * 6], dt.bfloat16)
            ident = big.tile([128, 128], dt.float32)
            rgb_f8 = big.tile([128, 2, PH], dt.float8e4)
            ir_f8 = big.tile([128, 2, HW], dt.float8e4)
            scales = big.tile([128, NPT], dt.float32)
            S_stat = big.tile([128, NPT * NQB], dt.float32)
            M_stat = big.tile([128, NPT * NQB], dt.float32)
            C_sb = big.tile([6, HW], dt.float32)
            junk = big.tile([128, QB], dt.bfloat16)
            ones_f = big.tile([128, 1], dt.float32)
            ones_bf = big.tile([128, 1], dt.bfloat16)
            lnrs_t = big.tile([128, 1], dt.float32)
            lnirs_t = big.tile([128, 1], dt.float32)

            # ---------------- input DMAs (issue order matters) --------
            rgb_segs = []
            for s in range(PH // 512):
                seg = ldr.tile([128, 2 * 512], dt.float32, tag="rseg")
                nc.sync.dma_start(seg[:].rearrange("p (c m) -> p c m", c=2),
                                  rgb_in[:, :, s * 512:(s + 1) * 512].rearrange("c p m -> p c m"))
                rgb_segs.append(seg)
            ir_segs = []
            for s in range(2):
                seg = ldi.tile([128, 2 * 512], dt.float32, tag="iseg")
                nc.sync.dma_start(seg[:].rearrange("p (c m) -> p c m", c=2),
                                  ir_in[:, :, s * 512:(s + 1) * 512].rearrange("c p m -> p c m"))
                ir_segs.append(seg)
            nc.sync.dma_start(im_b[:], im_in)
            nc.sync.dma_start(rm_c[:], rm_in)
            nc.sync.dma_start(oh_b[:], oh_in)
            nc.sync.dma_start(ident[:], id_in)
            for s in range(2, NSEG):
                seg = ldi.tile([128, 2 * 512], dt.float32, tag="iseg")
                nc.sync.dma_start(seg[:].rearrange("p (c m) -> p c m", c=2),
                                  ir_in[:, :, s * 512:(s + 1) * 512].rearrange("c p m -> p c m"))
                ir_segs.append(seg)

            nc.vector.memset(ones_f[:], 1.0)
            nc.vector.tensor_copy(ones_bf[:], ones_f[:])
            nc.vector.memset(lnrs_t[:], LN_RS)
            nc.vector.memset(lnirs_t[:], LN_IRS)

            # ---------------- rgb prologue ----------------
            # cast raw f32 -> fp8 on ACT (idle this early), square on DVE,
            # window matmuls give ||rgb_p||^2 with p on partitions.
            nT_rgb = big.tile([128, NPT], dt.float32)
            for s in range(PH // 512):
                seg = rgb_segs[s]
                nc.scalar.activation(rgb_f8[:, :, s * 512:(s + 1) * 512],
                                     seg[:].rearrange("p (c m) -> p c m", c=2),
                                     AF.Copy)
                sq = scr.tile([128, 1024], dt.bfloat16, tag="sqr")
                nc.vector.tensor_mul(sq[:], rgb_f8[:, :, s * 512:(s + 1) * 512],
                                     rgb_f8[:, :, s * 512:(s + 1) * 512])
                pnT_r = psN.tile([128, 4], dt.float32, tag="pnT", name=f"pnTr{s}")
                for w in range(4):
                    for c in range(2):
                        nc.tensor.matmul(pnT_r[:, w:w + 1],
                                         sq[:, c * 512 + w * 128: c * 512 + (w + 1) * 128],
                                         ones_bf[:], start=(c == 0), stop=(c == 1))
                nc.vector.tensor_scalar_max(nT_rgb[:, s * 4:(s + 1) * 4], pnT_r[:],
                                            EPS_NSQ)
            nL_rgb = rows.tile([128, NPT], dt.float32, tag="nLR")
            nc.scalar.activation(nL_rgb[:], nT_rgb[:], AF.Ln)
            # scales = (20/IRS) / ||rgb_p||
            nc.scalar.activation(scales[:], nL_rgb[:], AF.Exp,
                                 scale=-0.5, bias=lnrs_t[:])

            # ---------------- ir seg prep ----------------
            def prep_ir_seg(s, on_gps):
                seg = ir_segs[s]
                sqe = nc.gpsimd if on_gps else nc.vector
                mue = nc.gpsimd if on_gps else nc.vector
                sq = scr.tile([128, 1024], dt.bfloat16, tag="sqi")
                sqe.tensor_mul(sq[:], seg[:], seg[:])
                pnT = psN.tile([128, 4], dt.float32, tag="pnT", name=f"pnTi{s}")
                for w in range(4):
                    for c in range(2):
                        nc.tensor.matmul(pnT[:, w:w + 1],
                                         sq[:, c * 512 + w * 128: c * 512 + (w + 1) * 128],
                                         ones_bf[:], start=(c == 0), stop=(c == 1))
                nT = rows.tile([128, 4], dt.float32, tag="nT")
                nc.vector.tensor_scalar_max(nT[:], pnT[:], EPS_NSQ)
                nL = rows.tile([128, 4], dt.float32, tag="nL")
                nc.scalar.activation(nL[:], nT[:], AF.Ln)
                nD = rows.tile([128, 4], dt.float32, tag="nD")
                # nD = IRS / ||ir_q||  (per q, q on partitions in windows)
                nc.scalar.activation(nD[:], nL[:], AF.Exp, scale=-0.5,
                                     bias=lnirs_t[:])
                tT = psN.tile([1, 512], dt.float32, tag="tT", name=f"tT{s}")
                for w in range(4):
                    nc.tensor.matmul(tT[:, w * 128:(w + 1) * 128],
                                     nD[:, w:w + 1], ident[:], is_transpose=True)
                r4 = rows.tile([1, 512], dt.float32, tag="r4")
                nc.vector.tensor_copy(r4[:], tT[:])
                nb = scr.tile([128, 512], dt.float32, tag="nb")
                nc.gpsimd.partition_broadcast(nb[:], r4[:1, :])
                for c in range(2):
                    mue.tensor_mul(ir_f8[:, c, s * 512:(s + 1) * 512],
                                   seg[:, c * 512:(c + 1) * 512], nb[:])

            prep_ir_seg(0, on_gps=False)
            prep_ir_seg(1, on_gps=False)

            # ---------------- main loop ----------------
            pending = []
            psC_cur = [None]

            def flush_one():
                e_prev, qb0, pt0 = pending.pop(0)
                if pt0 == 0:
                    psC_cur[0] = psCp.tile([6, QB], dt.float32, tag="psC",
                                           name=f"psC{qb0}")
                psCq = psC_cur[0]
                for half in range(2):
                    nc.tensor.matmul(psCq[:, half * 512:(half + 1) * 512],
                                     oh_b[:, pt0 * 6:(pt0 + 1) * 6],
                                     e_prev[:, half * 512:(half + 1) * 512],
                                     start=(pt0 == 0), stop=(pt0 == NPT - 1))
                if pt0 == NPT - 1:
                    nc.vector.tensor_copy(C_sb[:, qb0 * QB:(qb0 + 1) * QB], psCq[:])

            for qb in range(NQB):
                for pt in range(NPT):
                    t = pt * NQB + qb
                    po = pt * 128
                    pl = psL.tile([128, QB], dt.float32, tag="pl")
                    for half in range(2):
                        qo = qb * QB + half * 512
                        nc.tensor.matmul(pl[:, half * 512:(half + 1) * 512],
                                         rgb_f8[:, :, po:po + 128],
                                         ir_f8[:, :, qo:qo + 512],
                                         start=True, stop=True, perf_mode=DR)
                    e_t = epool.tile([128, QB], dt.bfloat16, tag="e")
                    nc.scalar.activation(e_t[:], pl[:], AF.Exp,
                                         scale=scales[:, pt:pt + 1])
                    nc.vector.scalar_tensor_tensor(
                        out=junk[:],
                        in0=im_b[:, qb * QB:(qb + 1) * QB],
                        scalar=rm_c[:, pt:pt + 1],
                        in1=e_t[:],
                        op0=ALU.is_equal, op1=ALU.mult,
                        accum_out=M_stat[:, t:t + 1])
                    nc.vector.tensor_scalar(
                        junk[:], e_t[:], 1.0, 0.0, ALU.mult, ALU.add,
                        accum_out=S_stat[:, t:t + 1])
                    pending.append((e_t, qb, pt))
                    if len(pending) > DEFER:
                        flush_one()
                    # stream next-qb ir segs through mid-loop (gps path)
                    if qb < NQB - 1:
                        if pt == 5:
                            prep_ir_seg(2 * qb + 2, on_gps=True)
                        elif pt == 10:
                            prep_ir_seg(2 * qb + 3, on_gps=True)
            while pending:
                flush_one()

            # ---------------- epilogue ----------------
            S_red = big.tile([128, NPT], dt.float32)
            nc.vector.reduce_sum(S_red[:],
                                 S_stat[:].rearrange("p (pt q) -> p pt q", q=NQB),
                                 axis=mybir.AxisListType.X)
            M_red = big.tile([128, NPT], dt.float32)
            nc.vector.reduce_sum(M_red[:],
                                 M_stat[:].rearrange("p (pt q) -> p pt q", q=NQB),
                                 axis=mybir.AxisListType.X)
            nc.sync.dma_start(S_out.rearrange("pt p -> p pt"), S_red[:])
            nc.sync.dma_start(M_out.rearrange("pt p -> p pt"), M_red[:])
            nc.sync.dma_start(C_out, C_sb[:])

    nc.compile()
    return nc


def _get_nc():
    global _CACHED_NC
    if _CACHED_NC is None:
        _CACHED_NC = build_nc()
    return _CACHED_NC


def _build_in_maps(np_inputs):
    rgb_map = np.asarray(np_inputs["rgb_map"], dtype=np.float32).reshape(N, C, HW)
    ir_map = np.asarray(np_inputs["ir_map"], dtype=np.float32).reshape(N, C, HW)
    rm = np.asarray(np_inputs["rgb_mask"]).reshape(N, HW)
    im = np.asarray(np_inputs["ir_mask"]).reshape(N, HW)
    rm_f = rm.astype(np.float32)
    im_bf = im.astype(ml_dtypes.bfloat16)
    ident = np.eye(128, dtype=np.float32)

    in_maps = []
    for core in range(8):
        n, h = core // 2, core % 2
        psl = slice(h * PH, (h + 1) * PH)
        rgb_half = np.ascontiguousarray(rgb_map[n, :, psl].reshape(2, 128, PH))
        ir_full = np.ascontiguousarray(ir_map[n].reshape(2, 128, HW))
        im_bc = np.broadcast_to(im_bf[n], (128, HW)).copy()
        rm_half = rm_f[n, psl]
        rm_cols = np.ascontiguousarray(rm_half.reshape(NPT, 128).T)
        oh = np.empty((NPT, 128, 6), dtype=np.float32)
        oh[:, :, 0] = 1.0
        rm_tiles = rm_half.reshape(NPT, 128)
        for k in range(NCLS):
            oh[:, :, 1 + k] = (rm_tiles == k)
        oh_lhsT = np.ascontiguousarray(
            oh.transpose(1, 0, 2).reshape(128, NPT * 6)).astype(ml_dtypes.bfloat16)
        in_maps.append({
            "rgb_half": rgb_half,
            "ir_full": ir_full,
            "im_bcast": im_bc,
            "rm_cols": rm_cols,
            "oh_lhsT": oh_lhsT,
            "ident": ident,
        })
    return in_maps


def kernel(rgb_map, ir_map, rgb_mask, ir_mask):
    np_inputs = {"rgb_map": rgb_map, "ir_map": ir_map,
                 "rgb_mask": rgb_mask, "ir_mask": ir_mask}
    in_maps = _build_in_maps(np_inputs)
    im = np.asarray(ir_mask).reshape(N, HW)
    rm = np.asarray(rgb_mask).reshape(N, HW)

    nc = _get_nc()
    res = run_bass_kernel_spmd(nc, in_maps, list(range(8)))

    # ---------------- host combine (tiny) ----------------
    entries = []
    for n in range(N):
        rA, rB = res.results[2 * n], res.results[2 * n + 1]
        S = np.concatenate([rA["S_out"].reshape(PH), rB["S_out"].reshape(PH)]).astype(np.float64)
        M = np.concatenate([rA["M_out"].reshape(PH), rB["M_out"].reshape(PH)]).astype(np.float64)
        C6 = rA["C_out"].astype(np.float64) + rB["C_out"].astype(np.float64)
        Ce = C6[0]
        imn = im[n]
        Mc = C6[1 + imn, np.arange(HW)]
        r_rgb = (M / (S + EPS_DEN)) * (rm[n] > 0)
        r_ir = (Mc / (Ce + EPS_DEN)) * (imn > 0)
        entries.append(r_rgb)
        entries.append(r_ir)
    L = np.concatenate(entries)
    nz = L != 0
    total = -np.log(L[nz]).sum() if nz.any() else 0.0
    count = max(float(nz.sum()), 1.0)
    return np.asarray(np.float32(total / count))


if __name__ == "__main__":
    import reference
    inputs = reference.setup_inputs()
    inputs = {k: np.asarray(v) for k, v in inputs.items()}
    out = kernel(**inputs)
    print("kernel:", out)


# revision 16
# speedup vs baseline: 1.0685x; 1.0685x over previous
"""CrossPixContrastive loss on 8 trn2 NeuronCores.

Math (per batch n, HW=4096, C=256):
  rgb_n = l2norm_C(rgb); ir_n = l2norm_C(ir)
  e[p,q] = exp(20 * clip(<rgb_n[:,p], ir_n[:,q]>, -1, 1))
  S[p] = sum_q e ; M[p] = sum_q e * (rm_p == im_q)
  C[q] = sum_p e ; Mc[q] = sum_p e * (rm_p == im_q)
  r_rgb = M/(S+1e-6) ; r_ir = Mc/(C+1e-6)
  loss = mean(-log over nonzero of concat(r_rgb, r_ir) * fg)

Sharding: 8 cores = 4 batches x 2 halves of the rgb-pixel axis p.
Per-core tiling: [128p x 1024q] tiles of e.
  PE  : fp8(e4m3) DoubleRow matmuls (both 128-c chunks in one instr,
        0.5 cyc/row) for the logits; bf16 one-hot column-sum matmuls;
        window matmuls + column transposes for the norms
  ACT : e = Exp(scale_p * logit) -> bf16, scale_p = (20/16)/||rgb_p||
  DVE : masked accum (im==rm)*e -> M and plain accum e -> S, both at
        the 4x bf16 perf mode; psum drains
  GPS : ir-seg squares/broadcast/normalize-to-fp8 for streamed segs
ir is pre-scaled by 16/||ir_q|| into fp8; rgb is cast raw to fp8 and
its norm is folded into the per-partition ACT scale.
Host combines the tiny per-core partials into the scalar loss.
"""
import numpy as np
import ml_dtypes

import concourse.bacc as bacc
import concourse.tile as tile
from concourse import mybir
from concourse.bass_utils import run_bass_kernel_spmd

dt = mybir.dt
AF = mybir.ActivationFunctionType
ALU = mybir.AluOpType
DR = mybir.MatmulPerfMode.DoubleRow

N, C, H, W = 4, 256, 64, 64
HW = H * W                      # 4096
PH = HW // 2                    # 2048  p-half per core
NPT = PH // 128                 # 16    p-tiles
QB = 1024                       # q big-chunk
NQB = HW // QB                  # 4
NSEG = HW // 512                # 8 ir segs
NCLS = 5
IRS = 16.0                      # fp8 pre-scale on normalized ir
LN_IRS = float(np.log(IRS))
LN_RS = float(np.log(20.0 / IRS))
EPS_DEN = 1e-6
EPS_NSQ = 1e-24                 # clamp on ||x||^2  (=(1e-12)^2)
DEFER = 3                       # col-matmul deferral (tiles)

_CACHED_NC = None

_TABLES_PATCHED = False


def _patch_activation_tables():
    """Keep Exp/Ln only in natural_log_exp_and_others so the compiler
    loads one ACT table set instead of thrashing between exp/ln sets."""
    global _TABLES_PATCHED
    if _TABLES_PATCHED:
        return
    _TABLES_PATCHED = True
    import concourse.hw_specs as hw_specs
    import concourse.bacc as _bacc
    orig = hw_specs.get_activation_tables

    def patched(arch):
        tabs = {k: set(v) for k, v in orig(arch).items()}
        exp, ln = AF.Exp, AF.Ln
        for name, fns in tabs.items():
            if name != "natural_log_exp_and_others":
                fns.discard(exp)
                fns.discard(ln)
        return tabs

    hw_specs.get_activation_tables = patched
    if getattr(_bacc, "get_activation_tables", None) is orig:
        _bacc.get_activation_tables = patched


def build_nc():
    _patch_activation_tables()
    nc = bacc.Bacc("TRN2", target_bir_lowering=False, debug=False, num_devices=8)

    rgb_in = nc.dram_tensor("rgb_half", [2, 128, PH], dt.float32, kind="ExternalInput").ap()
    ir_in = nc.dram_tensor("ir_full", [2, 128, HW], dt.float32, kind="ExternalInput").ap()
    im_in = nc.dram_tensor("im_bcast", [128, HW], dt.bfloat16, kind="ExternalInput").ap()
    rm_in = nc.dram_tensor("rm_cols", [128, NPT], dt.bfloat16, kind="ExternalInput").ap()
    oh_in = nc.dram_tensor("oh_lhsT", [128, NPT * 6], dt.bfloat16, kind="ExternalInput").ap()
    id_in = nc.dram_tensor("ident", [128, 128], dt.float32, kind="ExternalInput").ap()

    S_out = nc.dram_tensor("S_out", [NPT, 128], dt.float32, kind="ExternalOutput").ap()
    M_out = nc.dram_tensor("M_out", [NPT, 128], dt.float32, kind="ExternalOutput").ap()
    C_out = nc.dram_tensor("C_out", [6, HW], dt.float32, kind="ExternalOutput").ap()

    with tile.TileContext(nc) as tc:
        with tc.tile_pool(name="big", bufs=1) as big, \
             tc.tile_pool(name="ldr", bufs=2) as ldr, \
             tc.tile_pool(name="ldi", bufs=4) as ldi, \
             tc.tile_pool(name="scr", bufs=2) as scr, \
             tc.tile_pool(name="rows", bufs=2) as rows, \
             tc.tile_pool(name="epool", bufs=6) as epool, \
             tc.tile_pool(name="psL", bufs=2, space="PSUM") as psL, \
             tc.tile_pool(name="psC", bufs=1, space="PSUM") as psCp, \
             tc.tile_pool(name="psN", bufs=1, space="PSUM") as psN:

            # ---------------- persistent tiles ----------------
            im_b = big.tile([128, HW], dt.bfloat16)
            rm_c = big.tile([128, NPT], dt.bfloat16)
            oh_b = big.tile([128, NPT * 6], dt.bfloat16)
            ident = big.tile([128, 128], dt.float32)
            rgb_f8 = big.tile([128, 2, PH], dt.float8e4)
            ir_f8 = big.tile([128, 2, HW], dt.float8e4)
            scales = big.tile([128, NPT], dt.float32)
            S_stat = big.tile([128, NPT * NQB], dt.float32)
            M_stat = big.tile([128, NPT * NQB], dt.float32)
            C_sb = big.tile([6, HW], dt.float32)
            junk = big.tile([128, QB], dt.bfloat16)
            ones_f = big.tile([128, 1], dt.float32)
            ones_bf = big.tile([128, 1], dt.bfloat16)
            lnrs_t = big.tile([128, 1], dt.float32)
            lnirs_t = big.tile([128, 1], dt.float32)

            # ---------------- input DMAs (issue order matters) --------
            rgb_segs = []
            for s in range(PH // 512):
                seg = ldr.tile([128, 2 * 512], dt.float32, tag="rseg")
                nc.sync.dma_start(seg[:].rearrange("p (c m) -> p c m", c=2),
                                  rgb_in[:, :, s * 512:(s + 1) * 512].rearrange("c p m -> p c m"))
                rgb_segs.append(seg)
            ir_segs = []
            for s in range(2):
                seg = ldi.tile([128, 2 * 512], dt.float32, tag="iseg")
                nc.sync.dma_start(seg[:].rearrange("p (c m) -> p c m", c=2),
                                  ir_in[:, :, s * 512:(s + 1) * 512].rearrange("c p m -> p c m"))
                ir_segs.append(seg)
            nc.sync.dma_start(im_b[:], im_in)
            nc.sync.dma_start(rm_c[:], rm_in)
            nc.sync.dma_start(oh_b[:], oh_in)
            nc.sync.dma_start(ident[:], id_in)
            for s in range(2, NSEG):
                seg = ldi.tile([128, 2 * 512], dt.float32, tag="iseg")
                nc.sync.dma_start(seg[:].rearrange("p (c m) -> p c m", c=2),
                                  ir_in[:, :, s * 512:(s + 1) * 512].rearrange("c p m -> p c m"))
                ir_segs.append(seg)

            nc.vector.memset(ones_f[:], 1.0)
            nc.vector.tensor_copy(ones_bf[:], ones_f[:])
            nc.vector.memset(lnrs_t[:], LN_RS)
            nc.vector.memset(lnirs_t[:], LN_IRS)

            # ---------------- rgb prologue ----------------
            # cast raw f32 -> fp8 on ACT (idle this early), square on DVE,
            # window matmuls give ||rgb_p||^2 with p on partitions.
            nT_rgb = big.tile([128, NPT], dt.float32)
            for s in range(PH // 512):
                seg = rgb_segs[s]
                nc.scalar.activation(rgb_f8[:, :, s * 512:(s + 1) * 512],
                                     seg[:].rearrange("p (c m) -> p c m", c=2),
                                     AF.Copy)
                sq = scr.tile([128, 1024], dt.bfloat16, tag="sqr")
                nc.vector.tensor_mul(sq[:], rgb_f8[:, :, s * 512:(s + 1) * 512],
                                     rgb_f8[:, :, s * 512:(s + 1) * 512])
                pnT_r = psN.tile([128, 4], dt.float32, tag="pnT", name=f"pnTr{s}")
                for w in range(4):
                    for c in range(2):
                        nc.tensor.matmul(pnT_r[:, w:w + 1],
                                         sq[:, c * 512 + w * 128: c * 512 + (w + 1) * 128],
                                         ones_bf[:], start=(c == 0), stop=(c == 1))
                nc.vector.tensor_scalar_max(nT_rgb[:, s * 4:(s + 1) * 4], pnT_r[:],
                                            EPS_NSQ)
            nL_rgb = rows.tile([128, NPT], dt.float32, tag="nLR")
            nc.scalar.activation(nL_rgb[:], nT_rgb[:], AF.Ln)
            # scales = (20/IRS) / ||rgb_p||
            nc.scalar.activation(scales[:], nL_rgb[:], AF.Exp,
                                 scale=-0.5, bias=lnrs_t[:])

            # ---------------- ir seg prep ----------------
            def prep_ir_seg(s, on_gps):
                seg = ir_segs[s]
                sqe = nc.gpsimd if on_gps else nc.vector
                mue = nc.gpsimd if on_gps else nc.vector
                sq = scr.tile([128, 1024], dt.bfloat16, tag="sqi")
                sqe.tensor_mul(sq[:], seg[:], seg[:])
                pnT = psN.tile([128, 4], dt.float32, tag="pnT", name=f"pnTi{s}")
                for w in range(4):
                    for c in range(2):
                        nc.tensor.matmul(pnT[:, w:w + 1],
                                         sq[:, c * 512 + w * 128: c * 512 + (w + 1) * 128],
                                         ones_bf[:], start=(c == 0), stop=(c == 1))
                nT = rows.tile([128, 4], dt.float32, tag="nT")
                nc.vector.tensor_scalar_max(nT[:], pnT[:], EPS_NSQ)
                nL = rows.tile([128, 4], dt.float32, tag="nL")
                nc.scalar.activation(nL[:], nT[:], AF.Ln)
                nD = rows.tile([128, 4], dt.float32, tag="nD")
                # nD = IRS / ||ir_q||  (per q, q on partitions in windows)
                nc.scalar.activation(nD[:], nL[:], AF.Exp, scale=-0.5,
                                     bias=lnirs_t[:])
                tT = psN.tile([1, 512], dt.float32, tag="tT", name=f"tT{s}")
                for w in range(4):
                    nc.tensor.matmul(tT[:, w * 128:(w + 1) * 128],
                                     nD[:, w:w + 1], ident[:], is_transpose=True)
                r4 = rows.tile([1, 512], dt.float32, tag="r4")
                nc.vector.tensor_copy(r4[:], tT[:])
                nb = scr.tile([128, 512], dt.float32, tag="nb")
                nc.gpsimd.partition_broadcast(nb[:], r4[:1, :])
                for c in range(2):
                    mue.tensor_mul(ir_f8[:, c, s * 512:(s + 1) * 512],
                                   seg[:, c * 512:(c + 1) * 512], nb[:])

            prep_ir_seg(0, on_gps=False)
            prep_ir_seg(1, on_gps=False)

            # ---------------- main loop ----------------
            pending = []
            psC_cur = [None]

            def flush_one():
                e_prev, qb0, pt0 = pending.pop(0)
                if pt0 == 0:
                    psC_cur[0] = psCp.tile([6, QB], dt.float32, tag="psC",
                                           name=f"psC{qb0}")
                psCq = psC_cur[0]
                for half in range(2):
                    nc.tensor.matmul(psCq[:, half * 512:(half + 1) * 512],
                                     oh_b[:, pt0 * 6:(pt0 + 1) * 6],
                                     e_prev[:, half * 512:(half + 1) * 512],
                                     start=(pt0 == 0), stop=(pt0 == NPT - 1))
                if pt0 == NPT - 1:
                    nc.vector.tensor_copy(C_sb[:, qb0 * QB:(qb0 + 1) * QB], psCq[:])

            for qb in range(NQB):
                for pt in range(NPT):
                    t = pt * NQB + qb
                    po = pt * 128
                    pl = psL.tile([128, QB], dt.float32, tag="pl")
                    for half in range(2):
                        qo = qb * QB + half * 512
                        nc.tensor.matmul(pl[:, half * 512:(half + 1) * 512],
                                         rgb_f8[:, :, po:po + 128],
                                         ir_f8[:, :, qo:qo + 512],
                                         start=True, stop=True, perf_mode=DR)
                    e_t = epool.tile([128, QB], dt.bfloat16, tag="e")
                    nc.scalar.activation(e_t[:], pl[:], AF.Exp,
                                         scale=scales[:, pt:pt + 1],
                                         accum_out=S_stat[:, t:t + 1])
                    nc.vector.scalar_tensor_tensor(
                        out=junk[:],
                        in0=im_b[:, qb * QB:(qb + 1) * QB],
                        scalar=rm_c[:, pt:pt + 1],
                        in1=e_t[:],
                        op0=ALU.is_equal, op1=ALU.mult,
                        accum_out=M_stat[:, t:t + 1])
                    pending.append((e_t, qb, pt))
                    if len(pending) > DEFER:
                        flush_one()
                    # stream next-qb ir segs through mid-loop (gps path)
                    if qb < NQB - 1:
                        if pt == 5:
                            prep_ir_seg(2 * qb + 2, on_gps=True)
                        elif pt == 10:
                            prep_ir_seg(2 * qb + 3, on_gps=True)
            while pending:
                flush_one()

            # ---------------- epilogue ----------------
            S_red = big.tile([128, NPT], dt.float32)
            nc.vector.reduce_sum(S_red[:],
                                 S_stat[:].rearrange("p (pt q) -> p pt q", q=NQB),
                                 axis=mybir.AxisListType.X)
            M_red = big.tile([128, NPT], dt.float32)
            nc.vector.reduce_sum(M_red[:],
                                 M_stat[:].rearrange("p (pt q) -> p pt q", q=NQB),
                                 axis=mybir.AxisListType.X)
            nc.sync.dma_start(S_out.rearrange("pt p -> p pt"), S_red[:])
            nc.sync.dma_start(M_out.rearrange("pt p -> p pt"), M_red[:])
            nc.sync.dma_start(C_out, C_sb[:])

    nc.compile()
    return nc


def _get_nc():
    global _CACHED_NC
    if _CACHED_NC is None:
        _CACHED_NC = build_nc()
    return _CACHED_NC


def _build_in_maps(np_inputs):
    rgb_map = np.asarray(np_inputs["rgb_map"], dtype=np.float32).reshape(N, C, HW)
    ir_map = np.asarray(np_inputs["ir_map"], dtype=np.float32).reshape(N, C, HW)
    rm = np.asarray(np_inputs["rgb_mask"]).reshape(N, HW)
    im = np.asarray(np_inputs["ir_mask"]).reshape(N, HW)
    rm_f = rm.astype(np.float32)
    im_bf = im.astype(ml_dtypes.bfloat16)
    ident = np.eye(128, dtype=np.float32)

    in_maps = []
    for core in range(8):
        n, h = core // 2, core % 2
        psl = slice(h * PH, (h + 1) * PH)
        rgb_half = np.ascontiguousarray(rgb_map[n, :, psl].reshape(2, 128, PH))
        ir_full = np.ascontiguousarray(ir_map[n].reshape(2, 128, HW))
        im_bc = np.broadcast_to(im_bf[n], (128, HW)).copy()
        rm_half = rm_f[n, psl]
        rm_cols = np.ascontiguousarray(rm_half.reshape(NPT, 128).T).astype(
            ml_dtypes.bfloat16)
        oh = np.empty((NPT, 128, 6), dtype=np.float32)
        oh[:, :, 0] = 1.0
        rm_tiles = rm_half.reshape(NPT, 128)
        for k in range(NCLS):
            oh[:, :, 1 + k] = (rm_tiles == k)
        oh_lhsT = np.ascontiguousarray(
            oh.transpose(1, 0, 2).reshape(128, NPT * 6)).astype(ml_dtypes.bfloat16)
        in_maps.append({
            "rgb_half": rgb_half,
            "ir_full": ir_full,
            "im_bcast": im_bc,
            "rm_cols": rm_cols,
            "oh_lhsT": oh_lhsT,
            "ident": ident,
        })
    return in_maps


def kernel(rgb_map, ir_map, rgb_mask, ir_mask):
    np_inputs = {"rgb_map": rgb_map, "ir_map": ir_map,
                 "rgb_mask": rgb_mask, "ir_mask": ir_mask}
    in_maps = _build_in_maps(np_inputs)
    im = np.asarray(ir_mask).reshape(N, HW)
    rm = np.asarray(rgb_mask).reshape(N, HW)

    nc = _get_nc()
    res = run_bass_kernel_spmd(nc, in_maps, list(range(8)))

    # ---------------- host combine (tiny) ----------------
    entries = []
    for n in range(N):
        rA, rB = res.results[2 * n], res.results[2 * n + 1]
        S = np.concatenate([rA["S_out"].reshape(PH), rB["S_out"].reshape(PH)]).astype(np.float64)
        M = np.concatenate([rA["M_out"].reshape(PH), rB["M_out"].reshape(PH)]).astype(np.float64)
        C6 = rA["C_out"].astype(np.float64) + rB["C_out"].astype(np.float64)
        Ce = C6[0]
        imn = im[n]
        Mc = C6[1 + imn, np.arange(HW)]
        r_rgb = (M / (S + EPS_DEN)) * (rm[n] > 0)
        r_ir = (Mc / (Ce + EPS_DEN)) * (imn > 0)
        entries.append(r_rgb)
        entries.append(r_ir)
    L = np.concatenate(entries)
    nz = L != 0
    total = -np.log(L[nz]).sum() if nz.any() else 0.0
    count = max(float(nz.sum()), 1.0)
    return np.asarray(np.float32(total / count))


if __name__ == "__main__":
    import reference
    inputs = reference.setup_inputs()
    inputs = {k: np.asarray(v) for k, v in inputs.items()}
    out = kernel(**inputs)
    print("kernel:", out)
